# revision 1
# baseline (speedup 1.0000x reference)
import os
import sys

for _p in ("/opt/trn_rl_repo", os.path.expanduser("~/.axon_site/_ro/trn_rl_repo")):
    if os.path.isdir(_p) and _p not in sys.path:
        sys.path.insert(0, _p)

import numpy as np
import ml_dtypes

import concourse.bass as bass
from concourse import bacc
import concourse.tile as tile
import concourse.mybir as mybir
from concourse.bass_utils import run_bass_kernel_spmd

# Problem shape (hardcoded per contract)
B, T, D, H, DK = 4, 2048, 1024, 16, 64
NCORES = 8

# Sharding: core = (batch b, head-group hg). Each core handles 8 heads of one
# batch over the full sequence, row-shards W_o, and the host sums the two
# partial outputs per batch (the "all-reduce" of the tensor-parallel scheme).
HC = H // 2       # 8 heads per core
DC = HC * DK      # 512 hidden dims per core

P = 128
NDT = D // P      # 8 din tiles
NHT = DC // P     # 4 dout tiles for this core's heads
NKT = T // P      # 16 key-token tiles
NPAIR = HC // 2   # 4 head pairs (pair p <-> dout tile p)
QCH = 512         # free-dim chunk per matmul
NQC = T // QCH    # 4 q-chunks (all tokens are queries now)
NTT = T // P      # 16 output token tiles

bf16 = mybir.dt.bfloat16
f32 = mybir.dt.float32
FT = mybir.ActivationFunctionType
ADD = mybir.AluOpType.add
MUL = mybir.AluOpType.mult

_CACHE = {}


def build_kernel():
    nc = bacc.Bacc("TRN2", target_bir_lowering=False, debug=False, num_devices=1)

    # Per-core inputs (already sliced to this core's heads where applicable)
    xT = nc.dram_tensor("xT", [D, T], bf16, kind="ExternalInput")  # x[b].T
    # weights arrive pre-tiled from the host so every DMA is contiguous:
    # Wq/Wk: [dt, p, a, m] = W[a*128+p, dt*128+m]; Wv: [p, a, m] = W[a*128+p, m]
    # Wo: [ch, p, a, m] = Wo_shard[a*128+p, ch*512+m]
    Wq = nc.dram_tensor("Wq", [NHT, P, NDT, P], bf16, kind="ExternalInput")
    Wk = nc.dram_tensor("Wk", [NHT, P, NDT, P], bf16, kind="ExternalInput")
    Wv = nc.dram_tensor("Wv", [P, NDT, DC], bf16, kind="ExternalInput")
    Wo = nc.dram_tensor("Wo", [2, P, NHT, QCH], bf16, kind="ExternalInput")
    # bq/bk pre-striped on host to [128, NHT] (col t = bias[t*128:(t+1)*128])
    bqp = nc.dram_tensor("bqp", [P, NHT], f32, kind="ExternalInput")
    bkp = nc.dram_tensor("bkp", [P, NHT], f32, kind="ExternalInput")
    bv = nc.dram_tensor("bv", [1, DC], f32, kind="ExternalInput")
    bo = nc.dram_tensor("bo", [1, D], f32, kind="ExternalInput")  # pre-halved
    out = nc.dram_tensor("out", [T, D], mybir.dt.float16, kind="ExternalOutput")

    with tile.TileContext(nc) as tc:
        with (
            tc.tile_pool(name="big", bufs=1) as big,
            tc.tile_pool(name="tmp", bufs=3) as tmp,
            tc.tile_pool(name="res", bufs=4) as resp,
            tc.tile_pool(name="dram", bufs=1, space="DRAM") as dramp,
            tc.tile_pool(name="acc", bufs=2, space="PSUM") as accp,
            tc.tile_pool(name="sg", bufs=2, space="PSUM") as sgp,
            tc.tile_pool(name="ops", bufs=1, space="PSUM") as opsp,
        ):
            # K-projection weights prefetched first (first matmuls need them),
            # then x^T tiles quarter-major so the earliest columns land first.
            wk_w = big.tile([P, NHT, NDT, P], bf16, name="wk_w")
            for dt in range(NHT):
                nc.sync.dma_start(wk_w[:, dt], Wk[dt])
            xt_sb = [big.tile([P, T], bf16, name=f"xt{i}") for i in range(NDT)]
            wq_w = big.tile([P, NHT, NDT, P], bf16, name="wq_w")
            for q in range(4):
                sl = slice(q * QCH, (q + 1) * QCH)
                for i in range(NDT):
                    eng = nc.sync if i % 2 == 0 else nc.gpsimd
                    eng.dma_start(xt_sb[i][:, sl], xT[i * P : (i + 1) * P, sl])
            # Q weights after x but ahead of the larger deferred loads
            for dt in range(NHT):
                nc.gpsimd.dma_start(wq_w[:, dt], Wq[dt])

            bq_sb = big.tile([P, NHT], f32, name="bq_sb")
            bk_sb = big.tile([P, NHT], f32, name="bk_sb")
            nc.sync.dma_start(bq_sb[:], bqp[:])
            nc.sync.dma_start(bk_sb[:], bkp[:])
            bv_rep = big.tile([P, DC], f32, name="bv_rep")
            bo_rep = big.tile([P, D], f32, name="bo_rep")
            wv_ch = big.tile([P, NDT, DC], bf16, name="wv_ch")
            wo_ch = [big.tile([P, NHT, QCH], bf16, name=f"wo{ch}") for ch in range(2)]

            # persistent activations
            kt_sb = [big.tile([P, T], bf16, name=f"kt{p}") for p in range(NPAIR)]
            qt_sb = [big.tile([P, T], bf16, name=f"qt{p}") for p in range(NPAIR)]
            vp_sb = [big.tile([P, HC, DK + 1], bf16, name=f"vp{t}") for t in range(NKT)]
            for t in range(NKT):
                nc.any.memset(vp_sb[t][:], 1.0)
            ob_sb = [
                [big.tile([P, QCH], bf16, name=f"ob{p}_{c}") for c in range(NQC)]
                for p in range(NPAIR)
            ]
            den_sb = [big.tile([HC, QCH], f32, name=f"den{c}") for c in range(NQC)]
            rec_dr = [dramp.tile([HC, QCH], f32, name=f"recd{c}") for c in range(NQC)]

            # ---------- phase 1: projections ----------
            def proj_tile(w_t, bias_sb, dst_tiles, dt, ch):
                ps = accp.tile([P, QCH], f32, name="proj_ps")
                for di in range(NDT):
                    nc.tensor.matmul(
                        ps[:],
                        w_t[:, di, :],
                        xt_sb[di][:, ch * QCH : (ch + 1) * QCH],
                        start=(di == 0),
                        stop=(di == NDT - 1),
                    )
                nc.vector.tensor_tensor(
                    dst_tiles[dt][:, ch * QCH : (ch + 1) * QCH],
                    ps[:],
                    bias_sb[:, dt : dt + 1].to_broadcast((P, QCH)),
                    ADD,
                )

            # K projection chunk-major: all dout tiles of x-quarter q before
            # quarter q+1 is needed (wk_w is fully prefetched)
            for ch in range(NQC):
                for dt in range(NHT):
                    proj_tile(wk_w[:, dt], bk_sb, kt_sb, dt, ch)
            # staged loads deferred so the first projections' DMAs go first
            nc.sync.dma_start(wv_ch[:], Wv[:])
            nc.sync.dma_start(bv_rep[:], bv[:].to_broadcast((P, DC)))
            nc.sync.dma_start(bo_rep[:], bo[:].to_broadcast((P, D)))
            for ch in range(2):
                nc.sync.dma_start(wo_ch[ch][:], Wo[ch])
            # V in natural layout, scattered into the padded V' tiles
            # (before Q: the attention-output matmuls need all V tiles, while
            # scores only need the Q tile of their own pair)
            for tt in range(NKT):
                ps = accp.tile([P, QCH], f32, name="proj_ps")
                for di in range(NDT):
                    nc.tensor.matmul(
                        ps[:],
                        xt_sb[di][:, tt * P : (tt + 1) * P],
                        wv_ch[:, di, :],
                        start=(di == 0),
                        stop=(di == NDT - 1),
                    )
                # all heads laid out as [V(64) | 1]
                nc.vector.tensor_tensor(
                    vp_sb[tt][:, :, 0:DK],
                    ps[:].rearrange("p (h d) -> p h d", d=DK),
                    bv_rep[:].rearrange("p (h d) -> p h d", d=DK),
                    ADD,
                )

            for ch in range(NQC):
                for dt in range(NHT):
                    proj_tile(wq_w[:, dt], bq_sb, qt_sb, dt, ch)

            # ---------- phase 2: attention ----------
            for c in range(NQC):
                qsl = slice(c * QCH, (c + 1) * QCH)
                for p in range(NPAIR):
                    hA, hB = 2 * p, 2 * p + 1
                    oA = opsp.tile([P, QCH], f32, name="oA")
                    oB = opsp.tile([P, QCH], f32, name="oB")
                    for g in range(NKT // 2):
                        sgA = sgp.tile([P, 2, QCH], f32, tag="sg")
                        sgB = sgp.tile([P, 2, QCH], f32, tag="sg")
                        for j in range(2):
                            kt = 2 * g + j
                            ksl = slice(kt * P, (kt + 1) * P)
                            nc.tensor.matmul(
                                sgA[:, j, :],
                                kt_sb[p][0:DK, ksl],
                                qt_sb[p][0:DK, qsl],
                                start=True,
                                stop=True,
                                tile_position=(0, 0),
                            )
                            nc.tensor.matmul(
                                sgB[:, j, :],
                                kt_sb[p][DK:P, ksl],
                                qt_sb[p][DK:P, qsl],
                                start=True,
                                stop=True,
                                tile_position=(64, 0),
                            )
                        ptA = tmp.tile([P, 2, QCH], bf16, tag="pt")
                        ptB = tmp.tile([P, 2, QCH], bf16, tag="pt")
                        nc.scalar.activation(ptA[:], sgA[:], FT.Exp, scale=0.125)
                        nc.scalar.activation(ptB[:], sgB[:], FT.Exp, scale=0.125)
                        for j in range(2):
                            kt = 2 * g + j
                            nc.tensor.matmul(
                                oA[0:65, :],
                                vp_sb[kt][:, hA, :],
                                ptA[:, j, :],
                                start=(kt == 0),
                                stop=(kt == NKT - 1),
                            )
                            nc.tensor.matmul(
                                oB[0:65, :],
                                vp_sb[kt][:, hB, :],
                                ptB[:, j, :],
                                start=(kt == 0),
                                stop=(kt == NKT - 1),
                            )
                    # raw O^T to SBUF (bf16); head B via staging + shift DMA.
                    # Denominators (row 64) stage through fp32 row tiles.
                    nc.vector.tensor_copy(ob_sb[p][c][0:DK, :], oA[0:DK, :])
                    stgB = tmp.tile([DK, QCH], bf16, tag="bstg")
                    nc.vector.tensor_copy(stgB[:], oB[0:DK, :])
                    nc.gpsimd.dma_start(ob_sb[p][c][DK:P, :], stgB[:])
                    stgDA = tmp.tile([65, QCH], f32, tag="dstgA")
                    stgDB = tmp.tile([65, QCH], f32, tag="dstgB")
                    nc.vector.tensor_copy(stgDA[64:65, :], oA[64:65, :])
                    nc.vector.tensor_copy(stgDB[64:65, :], oB[64:65, :])
                    nc.gpsimd.dma_start(den_sb[c][hA : hA + 1, :], stgDA[64:65, :])
                    nc.gpsimd.dma_start(den_sb[c][hB : hB + 1, :], stgDB[64:65, :])

                # normalize: reciprocal (in place), DRAM-bounced broadcast
                nc.vector.reciprocal(den_sb[c][:], den_sb[c][:])
                nc.gpsimd.dma_start(rec_dr[c][:], den_sb[c][:])
                for p in range(NPAIR):
                    hA, hB = 2 * p, 2 * p + 1
                    rep = tmp.tile([P, QCH], f32, tag="rep")
                    nc.gpsimd.dma_start(
                        rep[0:DK, :], rec_dr[c][hA : hA + 1, :].to_broadcast((DK, QCH))
                    )
                    nc.gpsimd.dma_start(
                        rep[DK:P, :], rec_dr[c][hB : hB + 1, :].to_broadcast((DK, QCH))
                    )
                    nc.vector.tensor_tensor(
                        ob_sb[p][c][:], ob_sb[p][c][:], rep[:], MUL
                    )

            # ---------- phase 3: output projection (row-sharded W_o) ----------
            # Emit chunk-3 token tiles last: their ob tiles normalize at the
            # very end of phase 2, and the scheduler follows emission order.
            out_v = out[:].rearrange("(tt p) d -> p tt d", p=P)

            def out_group(ch, tg):
                # 2 token tiles -> one grouped result tile -> one DMA
                res = resp.tile([P, 2, QCH], mybir.dt.float16, tag="ores")
                for k in range(2):
                    ttk = 2 * tg + k
                    c, s = ttk // 4, (ttk % 4) * P
                    ps = accp.tile([P, QCH], f32, name="proj_ps")
                    for p in range(NPAIR):
                        nc.tensor.matmul(
                            ps[:],
                            ob_sb[p][c][:, s : s + P],
                            wo_ch[ch][:, p, :],
                            start=(p == 0),
                            stop=(p == NPAIR - 1),
                        )
                    nc.vector.tensor_tensor(
                        res[:, k, :], ps[:], bo_rep[:, ch * QCH : (ch + 1) * QCH], ADD
                    )
                nc.sync.dma_start(
                    out_v[:, 2 * tg : 2 * tg + 2, ch * QCH : (ch + 1) * QCH],
                    res[:],
                )

            for ch in range(2):
                for tg in range(6):
                    out_group(ch, tg)
            for ch in range(2):
                for tg in range(6, 8):
                    out_group(ch, tg)

    nc.compile()
    return nc


def _prep_inputs(x, Wq, bq, Wk, bk, Wv, bv, Wo, bo):
    """Shard + lay out inputs for the 8 cores (batch x head-group)."""
    x = np.asarray(x, dtype=np.float32)
    to_bf = lambda a: np.ascontiguousarray(a).astype(ml_dtypes.bfloat16)
    Wq, Wk, Wv, Wo = (np.asarray(w, np.float32) for w in (Wq, Wk, Wv, Wo))
    bq, bk, bv, bo = (np.asarray(v, np.float32) for v in (bq, bk, bv, bo))
    bo_half = np.ascontiguousarray((bo * 0.5).reshape(1, D))
    xTb = [to_bf(x[b].T) for b in range(B)]
    in_maps = []
    for core in range(NCORES):
        b, hg = core // 2, core % 2
        csl = slice(hg * DC, (hg + 1) * DC)

        def tile_qk(W):
            # [D, DC] -> [dt, p, a, m]
            return to_bf(
                W[:, csl].reshape(NDT, P, NHT, P).transpose(2, 1, 0, 3)
            )

        in_maps.append(
            {
                "xT": xTb[b],
                "Wq": tile_qk(Wq),
                "Wk": tile_qk(Wk),
                "Wv": to_bf(Wv[:, csl].reshape(NDT, P, DC).transpose(1, 0, 2)),
                "Wo": to_bf(
                    Wo[csl, :].reshape(NHT, P, 2, QCH).transpose(2, 1, 0, 3)
                ),
                "bqp": np.ascontiguousarray(bq[csl].reshape(NHT, P).T),
                "bkp": np.ascontiguousarray(bk[csl].reshape(NHT, P).T),
                "bv": np.ascontiguousarray(bv[csl].reshape(1, DC)),
                "bo": bo_half,
            }
        )
    return in_maps


def kernel(x, Wq, bq, Wk, bk, Wv, bv, Wo, bo):
    if "nc" not in _CACHE:
        _CACHE["nc"] = build_kernel()
    nc = _CACHE["nc"]
    in_maps = _prep_inputs(x, Wq, bq, Wk, bk, Wv, bv, Wo, bo)
    res = run_bass_kernel_spmd(nc, in_maps, list(range(NCORES)))
    out = np.empty((B, T, D), dtype=np.float32)
    for b in range(B):
        out[b] = res.results[2 * b]["out"].astype(np.float32) + res.results[
            2 * b + 1
        ]["out"].astype(np.float32)
    return out



# revision 11
# speedup vs baseline: 1.2799x; 1.2799x over previous
import os
import sys

for _p in ("/opt/trn_rl_repo", os.path.expanduser("~/.axon_site/_ro/trn_rl_repo")):
    if os.path.isdir(_p) and _p not in sys.path:
        sys.path.insert(0, _p)

import numpy as np
import ml_dtypes

import concourse.bass as bass
from concourse import bacc
import concourse.tile as tile
import concourse.mybir as mybir
from concourse.bass_utils import run_bass_kernel_spmd

# Problem shape (hardcoded per contract)
B, T, D, H, DK = 4, 2048, 1024, 16, 64
NCORES = 8

# Sharding: core = (batch b, head-group hg). Each core handles 8 heads of one
# batch over the full sequence, row-shards W_o, and the host sums the two
# partial outputs per batch (the "all-reduce" of the tensor-parallel scheme).
HC = H // 2       # 8 heads per core
DC = HC * DK      # 512 hidden dims per core

P = 128
NG = D // P       # 8 contraction tiles for the projections
NPAIR = HC // 2   # 4 Q/K projection tiles (2 heads each)
NKT = T // P      # 16 key-token tiles
QCH = 512         # query-chunk width
NQC = T // QCH    # 4 query chunks
NQB = QCH // P    # 4 query blocks of 128 per chunk
NGR = NKT // 2    # 8 score groups (2 key tiles each) per (chunk, head)

bf16 = mybir.dt.bfloat16
fp8 = mybir.dt.float8e4
f16 = mybir.dt.float16
f32 = mybir.dt.float32
i16 = mybir.dt.int16
FT = mybir.ActivationFunctionType
ADD = mybir.AluOpType.add
MUL = mybir.AluOpType.mult
DR = mybir.MatmulPerfMode.DoubleRow

LOG2E = 1.4426950408889634
# DVE fast-exp (int16 bitcast to bf16): i16 = round(s*ALPHA + BETA)
ALPHA = 0.125 * LOG2E * 128.0
BETA = 16256.0 - 0.5

# which of the 8 score groups per (c,h) use the DVE bit-trick exp
DVE_GROUPS = (2, 5)
# consume (attnV) emission trails the produce stream by this many steps
LAGU = 6

_CACHE = {}


def build_kernel():
    nc = bacc.Bacc("TRN2", target_bir_lowering=False, debug=False, num_devices=1)

    xT = nc.dram_tensor("xT", [NG, P, T], bf16, kind="ExternalInput")
    Wq = nc.dram_tensor("Wq", [P, NG, NPAIR, P], bf16, kind="ExternalInput")
    Wk = nc.dram_tensor("Wk", [P, NG, NPAIR, P], bf16, kind="ExternalInput")
    Wv = nc.dram_tensor("Wv", [P, NG, DC], bf16, kind="ExternalInput")
    Wo = nc.dram_tensor("Wo", [P, 4, D], bf16, kind="ExternalInput")
    bqp = nc.dram_tensor("bqp", [P, NPAIR], f32, kind="ExternalInput")
    bkp = nc.dram_tensor("bkp", [P, NPAIR], f32, kind="ExternalInput")
    bv = nc.dram_tensor("bv", [1, DC], f32, kind="ExternalInput")
    bo = nc.dram_tensor("bo", [1, D], f32, kind="ExternalInput")  # pre-halved
    iden = nc.dram_tensor("iden", [P, P], bf16, kind="ExternalInput")
    out = nc.dram_tensor("out", [T, D], f16, kind="ExternalOutput")

    with tile.TileContext(nc) as tc:
        with (
            tc.tile_pool(name="big", bufs=1) as big,
            tc.tile_pool(name="pt", bufs=20) as ptp,
            tc.tile_pool(name="stg", bufs=4) as stgp,
            tc.tile_pool(name="rc", bufs=4) as rcp,
            tc.tile_pool(name="res", bufs=4) as resp,
            tc.tile_pool(name="sg", bufs=2, space="PSUM") as sgp,
            tc.tile_pool(name="av", bufs=2, space="PSUM") as avp,
            tc.tile_pool(name="acc", bufs=2, space="PSUM") as accp,
        ):
            # ---------- prefetch ----------
            wk_sb = big.tile([P, NG, NPAIR, P], bf16, name="wk_sb")
            nc.sync.dma_start(wk_sb[:], Wk[:])
            xt_sb = [big.tile([P, T], bf16, name=f"xt{g}") for g in range(NG)]
            for g in range(NG):
                nc.sync.dma_start(xt_sb[g][:], xT[g])
            bk_sb = big.tile([P, NPAIR], f32, name="bk_sb")
            bq_sb = big.tile([P, NPAIR], f32, name="bq_sb")
            nc.sync.dma_start(bk_sb[:], bkp[:])
            nc.sync.dma_start(bq_sb[:], bqp[:])
            wq_sb = big.tile([P, NG, NPAIR, P], bf16, name="wq_sb")
            nc.sync.dma_start(wq_sb[:], Wq[:])
            iden_sb = big.tile([P, P], bf16, name="iden_sb")
            nc.sync.dma_start(iden_sb[:], iden[:])
            wv_sb = big.tile([P, NG, DC], bf16, name="wv_sb")
            nc.sync.dma_start(wv_sb[:], Wv[:])
            bv_rep = big.tile([P, DC], f32, name="bv_rep")
            nc.sync.dma_start(bv_rep[:], bv[:].to_broadcast((P, DC)))
            wo_sb = big.tile([P, 4, D], bf16, name="wo_sb")
            nc.sync.dma_start(wo_sb[:], Wo[:])
            bo_rep = big.tile([P, D], f32, name="bo_rep")
            nc.sync.dma_start(bo_rep[:], bo[:].to_broadcast((P, D)))

            # persistent activations: q/k in fp8 DoubleRow slab layout
            kq_sb = [big.tile([P, 2, T], fp8, name=f"kq{r}") for r in range(2)]
            qq_sb = [big.tile([P, 2, T], fp8, name=f"qq{r}") for r in range(2)]
            vp_sb = [big.tile([P, HC, DK + 1], bf16, name=f"vp{t}") for t in range(NKT)]
            for t in range(NKT):
                nc.any.memset(vp_sb[t][:], 1.0)
            ob_sb = [
                [big.tile([P, DC], bf16, name=f"ob{cb}_{qb}") for qb in range(NQB)]
                for cb in range(2)
            ]
            obT_sb = [
                [big.tile([P, QCH], bf16, name=f"obT{cb}_{ds}") for ds in range(4)]
                for cb in range(2)
            ]

            # ---------- projection emitters ----------
            def proj_qk(w_sb, bias_sb, dst, dt, c):
                ps = accp.tile([P, QCH], f32, tag="proj")
                for g in range(NG):
                    nc.tensor.matmul(
                        ps[:],
                        w_sb[:, g, dt, :],
                        xt_sb[g][:, c * QCH : (c + 1) * QCH],
                        start=(g == 0),
                        stop=(g == NG - 1),
                    )
                st = stgp.tile([P, QCH], fp8, tag="qkstg")
                nc.scalar.activation(
                    st[:], ps[:], FT.Identity, bias=bias_sb[:, dt : dt + 1]
                )
                r, half = dt // 2, dt % 2
                nc.sync.dma_start(
                    dst[r][64 * half : 64 * half + 64, :, c * QCH : (c + 1) * QCH],
                    st[:],
                )

            def proj_v(tt):
                ps = accp.tile([P, QCH], f32, tag="proj")
                for g in range(NG):
                    nc.tensor.matmul(
                        ps[:],
                        xt_sb[g][:, tt * P : (tt + 1) * P],
                        wv_sb[:, g, :],
                        start=(g == 0),
                        stop=(g == NG - 1),
                    )
                nc.vector.tensor_tensor(
                    vp_sb[tt][:, :, 0:DK],
                    ps[:].rearrange("p (h d) -> p h d", d=DK),
                    bv_rep[:].rearrange("p (h d) -> p h d", d=DK),
                    ADD,
                )

            # ---------- attention emitters ----------
            def emit_produce(c, h, g):
                r, j = h // 4, h % 4
                bsl = slice(32 * j, 32 * j + 32)
                qsl = slice(c * QCH, (c + 1) * QCH)
                sg = sgp.tile([P, 2, QCH], f32, tag="sg")
                for i in range(2):
                    kt = 2 * g + i
                    nc.tensor.matmul(
                        sg[:, i, :],
                        kq_sb[r][bsl, :, kt * P : (kt + 1) * P],
                        qq_sb[r][bsl, :, qsl],
                        start=True,
                        stop=True,
                        perf_mode=DR,
                        tile_position=(32 * j, 0),
                    )
                pt = ptp.tile([P, 2, QCH], bf16, tag="pt")
                if g in DVE_GROUPS:
                    nc.vector.tensor_scalar(
                        pt[:].bitcast(i16), sg[:], ALPHA, BETA, MUL, ADD
                    )
                else:
                    nc.scalar.activation(pt[:], sg[:], FT.Exp, scale=0.125)
                return pt

            def emit_consume(c, h, qb, pts):
                cb = c % 2
                av = avp.tile([P, QCH], f32, tag="av")
                for kt in range(NKT):
                    nc.tensor.matmul(
                        av[:, 0:65],
                        pts[kt // 2][:, kt % 2, qb * P : (qb + 1) * P],
                        vp_sb[kt][:, h, :],
                        start=(kt == 0),
                        stop=(kt == NKT - 1),
                    )
                rec = rcp.tile([P, 1], f32, tag="rec")
                nc.vector.reciprocal(rec[:], av[:, 64:65])
                nc.vector.tensor_tensor(
                    ob_sb[cb][qb][:, h * DK : (h + 1) * DK],
                    av[:, 0:DK],
                    rec[:].to_broadcast((P, DK)),
                    MUL,
                )

            def emit_transpose(c, ds):
                cb = c % 2
                trf = avp.tile([P, QCH], f32, tag="av")
                tr = trf[:].bitcast(bf16)
                for qb in range(NQB):
                    nc.tensor.transpose(
                        tr[:, qb * P : (qb + 1) * P],
                        ob_sb[cb][qb][:, ds * P : (ds + 1) * P],
                        iden_sb[:],
                    )
                nc.scalar.activation(obT_sb[cb][ds][:], tr[:, 0:QCH], FT.Copy)

            def emit_oproj(c, mb, tg):
                cb = c % 2
                msl = slice(mb * QCH, (mb + 1) * QCH)
                res = resp.tile([P, 2, QCH], f16, tag="ores")
                for k2 in range(2):
                    qb = 2 * tg + k2
                    ps = accp.tile([P, QCH], f32, tag="proj")
                    for ds in range(4):
                        nc.tensor.matmul(
                            ps[:],
                            obT_sb[cb][ds][:, qb * P : (qb + 1) * P],
                            wo_sb[:, ds, msl],
                            start=(ds == 0),
                            stop=(ds == 3),
                        )
                    nc.vector.tensor_tensor(res[:, k2, :], ps[:], bo_rep[:, msl], ADD)
                ov = out[:].rearrange("(a k p) m -> a p k m", k=2, p=P)
                nc.gpsimd.dma_start(ov[2 * c + tg, :, :, msl], res[:])

            # ---------- preamble: minimal work to start attention ----------
            for c in range(NQC):
                proj_qk(wk_sb, bk_sb, kq_sb, 0, c)
            proj_qk(wq_sb, bq_sb, qq_sb, 0, 0)

            # fillers injected into the produce stream: step -> [closures]
            fillers = {}

            def add_filler(pos, fn):
                fillers.setdefault(pos, []).append(fn)

            for g2 in range(NGR):
                add_filler(g2, lambda tt=2 * g2: proj_v(tt))
                add_filler(g2, lambda tt=2 * g2 + 1: proj_v(tt))
            for dt in range(1, NPAIR):
                base = 8 * dt
                add_filler(base, lambda d=dt: proj_qk(wk_sb, bk_sb, kq_sb, d, 0))
                add_filler(base + 1, lambda d=dt: proj_qk(wq_sb, bq_sb, qq_sb, d, 0))
                for c in range(1, NQC):
                    add_filler(
                        base + 1 + c, lambda d=dt, cc=c: proj_qk(wk_sb, bk_sb, kq_sb, d, cc)
                    )
            for c in range(1, NQC):
                for dt in range(NPAIR):
                    add_filler(
                        64 * (c - 1) + 32 + 4 * dt,
                        lambda d=dt, cc=c: proj_qk(wq_sb, bq_sb, qq_sb, d, cc),
                    )

            # ---------- main interleaved stream ----------
            stream = [
                (c, h, g) for c in range(NQC) for h in range(HC) for g in range(NGR)
            ]
            head_pts = {}
            consume_q = []   # (ready_pos, c, h, qb, pts)
            out_q = []       # (ready_pos, closure)
            t = 0

            def drain(pos, budget_consume=1, budget_out=1):
                nonlocal consume_q, out_q
                done_heads = []
                while consume_q and consume_q[0][0] <= pos and budget_consume > 0:
                    _, cc, hh, qb, pts = consume_q.pop(0)
                    emit_consume(cc, hh, qb, pts)
                    budget_consume -= 1
                    if qb == NQB - 1:
                        done_heads.append((cc, hh))
                while out_q and out_q[0][0] <= pos and budget_out > 0:
                    _, fn = out_q.pop(0)
                    fn()
                    budget_out -= 1
                return done_heads

            for t, (c, h, g) in enumerate(stream):
                for fn in fillers.pop(t, ()):
                    fn()
                pt = emit_produce(c, h, g)
                head_pts.setdefault((c, h), []).append(pt)
                if g == NGR - 1:
                    pts = head_pts.pop((c, h))
                    for qb in range(NQB):
                        consume_q.append((t + LAGU + qb, c, h, qb, pts))
                done = drain(t)
                for (cc, hh) in done:
                    if hh == HC - 1:
                        base = t + 2
                        for ds in range(4):
                            out_q.append((base + 2 * ds, lambda c2=cc, d=ds: emit_transpose(c2, d)))
                        k = 0
                        for mb in range(2):
                            for tg in range(2):
                                out_q.append(
                                    (base + 8 + 2 * k, lambda c2=cc, m=mb, t2=tg: emit_oproj(c2, m, t2))
                                )
                                k += 1

            # flush the tail
            pos = len(stream)
            while consume_q or out_q:
                pos += 1
                done = drain(pos, budget_consume=1, budget_out=1)
                for (cc, hh) in done:
                    if hh == HC - 1:
                        for ds in range(4):
                            out_q.append((pos, lambda c2=cc, d=ds: emit_transpose(c2, d)))
                        for mb in range(2):
                            for tg in range(2):
                                out_q.append((pos, lambda c2=cc, m=mb, t2=tg: emit_oproj(c2, m, t2)))

    nc.compile()
    return nc


def _prep_inputs(x, Wq, bq, Wk, bk, Wv, bv, Wo, bo):
    """Shard + lay out inputs for the 8 cores (batch x head-group)."""
    x = np.asarray(x, dtype=np.float32)
    to_bf = lambda a: np.ascontiguousarray(a).astype(ml_dtypes.bfloat16)
    Wq, Wk, Wv, Wo = (np.asarray(w, np.float32) for w in (Wq, Wk, Wv, Wo))
    bq, bk, bv, bo = (np.asarray(v, np.float32) for v in (bq, bk, bv, bo))
    bo_half = np.ascontiguousarray((bo * 0.5).reshape(1, D))
    iden = np.eye(P, dtype=ml_dtypes.bfloat16)
    xTb = [to_bf(x[b].T.reshape(NG, P, T)) for b in range(B)]
    # fp8-slab column permutation: staging partition p holds q-dim col(dt, p)
    m = np.arange(P)
    colperm = (m % 2) * 32 + (m // 2) % 32  # within 64-dim head block
    colidx = np.stack(
        [(2 * dt + (m // 2) // 32) * 64 + colperm for dt in range(NPAIR)]
    )  # [NPAIR, P]
    in_maps = []
    for core in range(NCORES):
        b, hg = core // 2, core % 2
        csl = slice(hg * DC, (hg + 1) * DC)

        def tile_qk(W):
            Wc = W[:, csl]  # [D, DC]
            # [p, g, dt, m] with permuted columns
            Wt = Wc[:, colidx.reshape(-1)].reshape(D, NPAIR, P)
            return to_bf(Wt.reshape(NG, P, NPAIR, P).transpose(1, 0, 2, 3))

        def stripe_bias(bvec):
            bc = bvec[csl]
            return np.ascontiguousarray(bc[colidx].T)  # [P, NPAIR]

        in_maps.append(
            {
                "xT": xTb[b],
                "Wq": tile_qk(Wq),
                "Wk": tile_qk(Wk),
                "Wv": to_bf(Wv[:, csl].reshape(NG, P, DC).transpose(1, 0, 2)),
                "Wo": to_bf(Wo[csl, :].reshape(4, P, D).transpose(1, 0, 2)),
                "bqp": stripe_bias(bq),
                "bkp": stripe_bias(bk),
                "bv": np.ascontiguousarray(bv[csl].reshape(1, DC)),
                "bo": bo_half,
                "iden": iden,
            }
        )
    return in_maps


def kernel(x, Wq, bq, Wk, bk, Wv, bv, Wo, bo):
    if "nc" not in _CACHE:
        _CACHE["nc"] = build_kernel()
    nc = _CACHE["nc"]
    in_maps = _prep_inputs(x, Wq, bq, Wk, bk, Wv, bv, Wo, bo)
    res = run_bass_kernel_spmd(nc, in_maps, list(range(NCORES)))
    out = np.empty((B, T, D), dtype=np.float32)
    for b in range(B):
        out[b] = res.results[2 * b]["out"].astype(np.float32) + res.results[
            2 * b + 1
        ]["out"].astype(np.float32)
    return out


# revision 17
# speedup vs baseline: 1.3183x; 1.0300x over previous
import os
import sys

for _p in ("/opt/trn_rl_repo", os.path.expanduser("~/.axon_site/_ro/trn_rl_repo")):
    if os.path.isdir(_p) and _p not in sys.path:
        sys.path.insert(0, _p)

import numpy as np
import ml_dtypes

import concourse.bass as bass
from concourse import bacc
import concourse.tile as tile
import concourse.mybir as mybir
from concourse.bass_utils import run_bass_kernel_spmd

# Problem shape (hardcoded per contract)
B, T, D, H, DK = 4, 2048, 1024, 16, 64
NCORES = 8

# Sharding: core = (batch b, head-group hg). Each core handles 8 heads of one
# batch over the full sequence, row-shards W_o, and the host sums the two
# partial outputs per batch (the "all-reduce" of the tensor-parallel scheme).
HC = H // 2       # 8 heads per core
DC = HC * DK      # 512 hidden dims per core

P = 128
NG = D // P       # 8 contraction tiles for the projections
NPAIR = HC // 2   # 4 Q/K projection tiles (2 heads each)
NKT = T // P      # 16 key-token tiles
QCH = 512         # query-chunk width
NQC = T // QCH    # 4 query chunks
NQB = QCH // P    # 4 query blocks of 128 per chunk
NGR = NKT // 2    # 8 score groups (2 key tiles each) per (chunk, head)

bf16 = mybir.dt.bfloat16
fp8 = mybir.dt.float8e4
f16 = mybir.dt.float16
f32 = mybir.dt.float32
i16 = mybir.dt.int16
FT = mybir.ActivationFunctionType
ADD = mybir.AluOpType.add
MUL = mybir.AluOpType.mult
DR = mybir.MatmulPerfMode.DoubleRow

LOG2E = 1.4426950408889634
# DVE fast-exp (int16 bitcast to bf16): i16 = round(s*ALPHA + BETA)
ALPHA = 0.125 * LOG2E * 128.0
BETA = 16256.0 - 0.5

# which of the 8 score groups per (c,h) use the DVE bit-trick exp
DVE_GROUPS = (2, 4, 6)
# consume (attnV) emission trails the produce stream by this many steps
LAGU = 6

_CACHE = {}


def build_kernel():
    nc = bacc.Bacc("TRN2", target_bir_lowering=False, debug=False, num_devices=1)

    xT = nc.dram_tensor("xT", [NG, P, T], bf16, kind="ExternalInput")
    Wq = nc.dram_tensor("Wq", [P, NG, NPAIR, P], bf16, kind="ExternalInput")
    Wk = nc.dram_tensor("Wk", [P, NG, NPAIR, P], bf16, kind="ExternalInput")
    Wv = nc.dram_tensor("Wv", [P, NG, DC], bf16, kind="ExternalInput")
    Wo = nc.dram_tensor("Wo", [P, 4, D], bf16, kind="ExternalInput")
    bqp = nc.dram_tensor("bqp", [P, NPAIR], f32, kind="ExternalInput")
    bkp = nc.dram_tensor("bkp", [P, NPAIR], f32, kind="ExternalInput")
    bv = nc.dram_tensor("bv", [1, DC], f32, kind="ExternalInput")
    bo = nc.dram_tensor("bo", [1, D], f32, kind="ExternalInput")  # pre-halved
    iden = nc.dram_tensor("iden", [P, P], bf16, kind="ExternalInput")
    out = nc.dram_tensor("out", [T, D], f16, kind="ExternalOutput")

    with tile.TileContext(nc) as tc:
        with (
            tc.tile_pool(name="big", bufs=1) as big,
            tc.tile_pool(name="pt", bufs=20) as ptp,
            tc.tile_pool(name="stg", bufs=4) as stgp,
            tc.tile_pool(name="rc", bufs=4) as rcp,
            tc.tile_pool(name="res", bufs=4) as resp,
            tc.tile_pool(name="sg", bufs=2, space="PSUM") as sgp,
            tc.tile_pool(name="av", bufs=2, space="PSUM") as avp,
            tc.tile_pool(name="acc", bufs=2, space="PSUM") as accp,
        ):
            # ---------- prefetch ----------
            wk_sb = big.tile([P, NG, NPAIR, P], bf16, name="wk_sb")
            nc.sync.dma_start(wk_sb[:], Wk[:])
            # x^T arrives in query-chunk column slices so the first K-proj
            # tiles can start ~3us in instead of waiting the full 8MB.
            xt_sb = [big.tile([P, T], bf16, name=f"xt{g}") for g in range(NG)]
            for g in range(NG):
                nc.sync.dma_start(xt_sb[g][:, 0:QCH], xT[g][:, 0:QCH])
            bk_sb = big.tile([P, NPAIR], f32, name="bk_sb")
            bq_sb = big.tile([P, NPAIR], f32, name="bq_sb")
            nc.sync.dma_start(bk_sb[:], bkp[:])
            nc.sync.dma_start(bq_sb[:], bqp[:])
            for g in range(NG):
                nc.sync.dma_start(xt_sb[g][:, QCH : 2 * QCH], xT[g][:, QCH : 2 * QCH])
            wq_sb = big.tile([P, NG, NPAIR, P], bf16, name="wq_sb")
            nc.sync.dma_start(wq_sb[:], Wq[:])
            iden_sb = big.tile([P, P], bf16, name="iden_sb")
            nc.sync.dma_start(iden_sb[:], iden[:])
            for g in range(NG):
                nc.sync.dma_start(
                    xt_sb[g][:, 2 * QCH : 3 * QCH], xT[g][:, 2 * QCH : 3 * QCH]
                )
            wv_sb = big.tile([P, NG, DC], bf16, name="wv_sb")
            nc.sync.dma_start(wv_sb[:], Wv[:])
            bv_rep = big.tile([P, DC], f32, name="bv_rep")
            nc.sync.dma_start(bv_rep[:], bv[:].to_broadcast((P, DC)))
            for g in range(NG):
                nc.sync.dma_start(
                    xt_sb[g][:, 3 * QCH : 4 * QCH], xT[g][:, 3 * QCH : 4 * QCH]
                )
            wo_sb = big.tile([P, 4, D], bf16, name="wo_sb")
            nc.sync.dma_start(wo_sb[:], Wo[:])
            bo_rep = big.tile([P, D], f32, name="bo_rep")
            nc.sync.dma_start(bo_rep[:], bo[:].to_broadcast((P, D)))

            # persistent activations: q/k in fp8 DoubleRow slab layout
            kq_sb = [big.tile([P, 2, T], fp8, name=f"kq{r}") for r in range(2)]
            qq_sb = [big.tile([P, 2, T], fp8, name=f"qq{r}") for r in range(2)]
            vp_sb = [big.tile([P, HC, DK + 1], bf16, name=f"vp{t}") for t in range(NKT)]
            for t in range(NKT):
                nc.any.memset(vp_sb[t][:], 1.0)
            ob_sb = [
                [big.tile([P, DC], bf16, name=f"ob{cb}_{qb}") for qb in range(NQB)]
                for cb in range(2)
            ]
            obT_sb = [
                [big.tile([P, QCH], bf16, name=f"obT{cb}_{ds}") for ds in range(4)]
                for cb in range(2)
            ]

            # ---------- projection emitters ----------
            # late_q: (pos, fn) conversions deferred a couple of produce steps
            # so they never park at the head of the in-order Act/DVE queues.
            late_q = []

            def proj_qk(w_sb, bias_sb, dst, dt, c, late_pos=None):
                ps = accp.tile([P, QCH], f32, tag="proj")
                for g in range(NG):
                    nc.tensor.matmul(
                        ps[:],
                        w_sb[:, g, dt, :],
                        xt_sb[g][:, c * QCH : (c + 1) * QCH],
                        start=(g == 0),
                        stop=(g == NG - 1),
                    )

                def finish(ps=ps, dt=dt, c=c):
                    st = stgp.tile([P, QCH], fp8, tag="qkstg")
                    nc.scalar.activation(
                        st[:], ps[:], FT.Identity, bias=bias_sb[:, dt : dt + 1]
                    )
                    r, half = dt // 2, dt % 2
                    nc.sync.dma_start(
                        dst[r][
                            64 * half : 64 * half + 64, :, c * QCH : (c + 1) * QCH
                        ],
                        st[:],
                    )

                if late_pos is None:
                    finish()
                else:
                    late_q.append((late_pos, finish))

            def proj_v(tt, late_pos=None):
                ps = accp.tile([P, QCH], f32, tag="proj")
                for g in range(NG):
                    nc.tensor.matmul(
                        ps[:],
                        xt_sb[g][:, tt * P : (tt + 1) * P],
                        wv_sb[:, g, :],
                        start=(g == 0),
                        stop=(g == NG - 1),
                    )

                def finish(ps=ps, tt=tt):
                    nc.vector.tensor_tensor(
                        vp_sb[tt][:, :, 0:DK],
                        ps[:].rearrange("p (h d) -> p h d", d=DK),
                        bv_rep[:].rearrange("p (h d) -> p h d", d=DK),
                        ADD,
                    )

                if late_pos is None:
                    finish()
                else:
                    late_q.append((late_pos, finish))

            # ---------- attention emitters ----------
            def emit_produce(c, h, g):
                r, j = h // 4, h % 4
                bsl = slice(32 * j, 32 * j + 32)
                qsl = slice(c * QCH, (c + 1) * QCH)
                sg = sgp.tile([P, 2, QCH], f32, tag="sg")
                for i in range(2):
                    kt = 2 * g + i
                    nc.tensor.matmul(
                        sg[:, i, :],
                        kq_sb[r][bsl, :, kt * P : (kt + 1) * P],
                        qq_sb[r][bsl, :, qsl],
                        start=True,
                        stop=True,
                        perf_mode=DR,
                        tile_position=(32 * j, 0),
                    )
                pt = ptp.tile([P, 2, QCH], bf16, tag="pt")
                if g in DVE_GROUPS:
                    nc.vector.tensor_scalar(
                        pt[:].bitcast(i16), sg[:], ALPHA, BETA, MUL, ADD
                    )
                else:
                    nc.scalar.activation(pt[:], sg[:], FT.Exp, scale=0.125)
                return pt

            def emit_consume(c, h, qb, pts):
                cb = c % 2
                av = avp.tile([P, QCH], f32, tag="av")
                for kt in range(NKT):
                    nc.tensor.matmul(
                        av[:, 0:65],
                        pts[kt // 2][:, kt % 2, qb * P : (qb + 1) * P],
                        vp_sb[kt][:, h, :],
                        start=(kt == 0),
                        stop=(kt == NKT - 1),
                    )
                rec = rcp.tile([P, 1], f32, tag="rec")
                nc.vector.reciprocal(rec[:], av[:, 64:65])
                nc.vector.tensor_tensor(
                    ob_sb[cb][qb][:, h * DK : (h + 1) * DK],
                    av[:, 0:DK],
                    rec[:].to_broadcast((P, DK)),
                    MUL,
                )

            def emit_transpose(c, ds):
                cb = c % 2
                trf = avp.tile([P, QCH], f32, tag="av")
                tr = trf[:].bitcast(bf16)
                for qb in range(NQB):
                    nc.tensor.transpose(
                        tr[:, qb * P : (qb + 1) * P],
                        ob_sb[cb][qb][:, ds * P : (ds + 1) * P],
                        iden_sb[:],
                    )
                nc.scalar.activation(obT_sb[cb][ds][:], tr[:, 0:QCH], FT.Copy)

            def emit_oproj(c, mb, tg):
                cb = c % 2
                msl = slice(mb * QCH, (mb + 1) * QCH)
                res = resp.tile([P, 2, QCH], f16, tag="ores")
                for k2 in range(2):
                    qb = 2 * tg + k2
                    ps = accp.tile([P, QCH], f32, tag="proj")
                    for ds in range(4):
                        nc.tensor.matmul(
                            ps[:],
                            obT_sb[cb][ds][:, qb * P : (qb + 1) * P],
                            wo_sb[:, ds, msl],
                            start=(ds == 0),
                            stop=(ds == 3),
                        )
                    nc.vector.tensor_tensor(res[:, k2, :], ps[:], bo_rep[:, msl], ADD)
                ov = out[:].rearrange("(a k p) m -> a p k m", k=2, p=P)
                nc.gpsimd.dma_start(ov[2 * c + tg, :, :, msl], res[:])

            # ---------- preamble: minimal work to start attention ----------
            for c in range(NQC):
                proj_qk(wk_sb, bk_sb, kq_sb, 0, c)
            proj_qk(wq_sb, bq_sb, qq_sb, 0, 0)

            # fillers injected into the produce stream: step -> [closures]
            fillers = {}

            def add_filler(pos, fn):
                fillers.setdefault(pos, []).append(fn)

            for g2 in range(NGR):
                add_filler(g2, lambda tt=2 * g2, p=g2: proj_v(tt, late_pos=p + 2))
                add_filler(g2, lambda tt=2 * g2 + 1, p=g2: proj_v(tt, late_pos=p + 2))
            for dt in range(1, NPAIR):
                base = 8 * dt
                add_filler(
                    base,
                    lambda d=dt, p=base: proj_qk(wk_sb, bk_sb, kq_sb, d, 0, p + 2),
                )
                add_filler(
                    base + 1,
                    lambda d=dt, p=base: proj_qk(wq_sb, bq_sb, qq_sb, d, 0, p + 3),
                )
                for c in range(1, NQC):
                    add_filler(
                        base + 1 + c,
                        lambda d=dt, cc=c, p=base + 1 + c: proj_qk(
                            wk_sb, bk_sb, kq_sb, d, cc, p + 2
                        ),
                    )
            for c in range(1, NQC):
                for dt in range(NPAIR):
                    add_filler(
                        64 * (c - 1) + 32 + 4 * dt,
                        lambda d=dt, cc=c, p=64 * (c - 1) + 32 + 4 * dt: proj_qk(
                            wq_sb, bq_sb, qq_sb, d, cc, p + 2
                        ),
                    )

            # ---------- main interleaved stream ----------
            stream = [
                (c, h, g) for c in range(NQC) for h in range(HC) for g in range(NGR)
            ]
            head_pts = {}
            consume_q = []   # (ready_pos, c, h, qb, pts)
            out_q = []       # (ready_pos, closure)
            t = 0

            def drain(pos, budget_consume=1, budget_out=1):
                nonlocal consume_q, out_q
                done_heads = []
                while consume_q and consume_q[0][0] <= pos and budget_consume > 0:
                    _, cc, hh, qb, pts = consume_q.pop(0)
                    emit_consume(cc, hh, qb, pts)
                    budget_consume -= 1
                    if qb == NQB - 1:
                        done_heads.append((cc, hh))
                while out_q and out_q[0][0] <= pos and budget_out > 0:
                    _, fn = out_q.pop(0)
                    fn()
                    budget_out -= 1
                return done_heads

            for t, (c, h, g) in enumerate(stream):
                while late_q and late_q[0][0] <= t:
                    late_q.pop(0)[1]()
                for fn in fillers.pop(t, ()):
                    fn()
                pt = emit_produce(c, h, g)
                head_pts.setdefault((c, h), []).append(pt)
                if g == NGR - 1:
                    pts = head_pts.pop((c, h))
                    for qb in range(NQB):
                        consume_q.append((t + LAGU + qb, c, h, qb, pts))
                done = drain(t)
                for (cc, hh) in done:
                    if hh == HC - 1:
                        base = t + 2
                        for ds in range(4):
                            out_q.append((base + 2 * ds, lambda c2=cc, d=ds: emit_transpose(c2, d)))
                        k = 0
                        for mb in range(2):
                            for tg in range(2):
                                out_q.append(
                                    (base + 8 + 2 * k, lambda c2=cc, m=mb, t2=tg: emit_oproj(c2, m, t2))
                                )
                                k += 1

            # flush the tail
            while late_q:
                late_q.pop(0)[1]()
            pos = len(stream)
            while consume_q or out_q:
                pos += 1
                done = drain(pos, budget_consume=1, budget_out=1)
                for (cc, hh) in done:
                    if hh == HC - 1:
                        for ds in range(4):
                            out_q.append((pos, lambda c2=cc, d=ds: emit_transpose(c2, d)))
                        for mb in range(2):
                            for tg in range(2):
                                out_q.append((pos, lambda c2=cc, m=mb, t2=tg: emit_oproj(c2, m, t2)))

    nc.compile()
    return nc


def _prep_inputs(x, Wq, bq, Wk, bk, Wv, bv, Wo, bo):
    """Shard + lay out inputs for the 8 cores (batch x head-group)."""
    x = np.asarray(x, dtype=np.float32)
    to_bf = lambda a: np.ascontiguousarray(a).astype(ml_dtypes.bfloat16)
    Wq, Wk, Wv, Wo = (np.asarray(w, np.float32) for w in (Wq, Wk, Wv, Wo))
    bq, bk, bv, bo = (np.asarray(v, np.float32) for v in (bq, bk, bv, bo))
    bo_half = np.ascontiguousarray((bo * 0.5).reshape(1, D))
    iden = np.eye(P, dtype=ml_dtypes.bfloat16)
    xTb = [to_bf(x[b].T.reshape(NG, P, T)) for b in range(B)]
    # fp8-slab column permutation: staging partition p holds q-dim col(dt, p)
    m = np.arange(P)
    colperm = (m % 2) * 32 + (m // 2) % 32  # within 64-dim head block
    colidx = np.stack(
        [(2 * dt + (m // 2) // 32) * 64 + colperm for dt in range(NPAIR)]
    )  # [NPAIR, P]
    in_maps = []
    for core in range(NCORES):
        b, hg = core // 2, core % 2
        csl = slice(hg * DC, (hg + 1) * DC)

        def tile_qk(W):
            Wc = W[:, csl]  # [D, DC]
            # [p, g, dt, m] with permuted columns
            Wt = Wc[:, colidx.reshape(-1)].reshape(D, NPAIR, P)
            return to_bf(Wt.reshape(NG, P, NPAIR, P).transpose(1, 0, 2, 3))

        def stripe_bias(bvec):
            bc = bvec[csl]
            return np.ascontiguousarray(bc[colidx].T)  # [P, NPAIR]

        in_maps.append(
            {
                "xT": xTb[b],
                "Wq": tile_qk(Wq),
                "Wk": tile_qk(Wk),
                "Wv": to_bf(Wv[:, csl].reshape(NG, P, DC).transpose(1, 0, 2)),
                "Wo": to_bf(Wo[csl, :].reshape(4, P, D).transpose(1, 0, 2)),
                "bqp": stripe_bias(bq),
                "bkp": stripe_bias(bk),
                "bv": np.ascontiguousarray(bv[csl].reshape(1, DC)),
                "bo": bo_half,
                "iden": iden,
            }
        )
    return in_maps


def kernel(x, Wq, bq, Wk, bk, Wv, bv, Wo, bo):
    if "nc" not in _CACHE:
        _CACHE["nc"] = build_kernel()
    nc = _CACHE["nc"]
    in_maps = _prep_inputs(x, Wq, bq, Wk, bk, Wv, bv, Wo, bo)
    res = run_bass_kernel_spmd(nc, in_maps, list(range(NCORES)))
    out = np.empty((B, T, D), dtype=np.float32)
    for b in range(B):
        out[b] = res.results[2 * b]["out"].astype(np.float32) + res.results[
            2 * b + 1
        ]["out"].astype(np.float32)
    return out
